# revision 31
# baseline (speedup 1.0000x reference)
"""Trainium2 Bass kernel for nn_DoubleAttention (causal attention + T5 relative
position bias), SPMD over 8 NeuronCores (head-parallel: core h owns head h).

Math notes that shape the kernel:
  * scores^T layout [s, q] (s on partitions) lets both attention matmuls run
    without any on-device transpose: QK^T uses host-pretransposed Q^T/K^T
    ([E, L] per head), and P^T is directly the rhs of the AV matmul with V in
    its natural [s, e] layout.
  * The T5 bias is Toeplitz: bias[l, s] = f(l - s), and the bucket function
    saturates at bucket 31 for distance >= 113.  Softmax is invariant to a
    per-row constant shift, so using exp(bias - bias_31) as a multiplicative
    factor makes the factor exactly 1.0 for distance >= 113.  Together with the
    causal mask this collapses to ONE [128, 256] band table applied to the
    first <=256 columns of each P^T tile.
  * Appending a ones-column to V makes the AV matmul also produce the softmax
    denominator (row 64 of the [65, L] accumulator); normalization happens on
    host, so no on-device reduction/divide at all.

Scheduling notes (walrus rejects matmuls carrying >1 semaphore wait):
  * Production AV matmuls are all start=False members of one accumulation
    group per PSUM bank (opened by a tiny start=True dummy per batch), so they
    carry no write-after-write completion self-waits.
  * Tiny "absorber" matmuls that genuinely read/write the tiles in question
    take each cross-engine wait first (absorbed waits update Tile's observed
    vector clock; nop waits do not):
      - dummy_q / dummy_v join the open bank-3 group and absorb the input-DMA
        waits (they accumulate lhsT*0 = +0, numerically harmless);
      - dummy_sp (per scores tile) takes the PSUM-slot WAR wait vs the exp
        that read the slot two tiles earlier;
      - dummy_tail (per batch) takes the completion tick of the batch's last
        AV matmul, covering the next batch's cross-generation writes;
      - dummy_war (per batch) takes the accumulator-WAR-vs-drain tick;
      - one DVE copy (wband) and one ACT exp (dummy_act) absorb ticks their
        engines would otherwise first observe mid-pipeline.
  * The banded product goes to its own tile (pband) so every AV matmul chunk
    reads a tile with exactly one producer (ACT or DVE, never both).
"""

import math

import numpy as np

import concourse.bass as bass
import concourse.mybir as mybir
import concourse.tile as tile
from concourse.bass_utils import run_bass_kernel_spmd
from concourse.tile_rust import add_dep_helper

B, L, H, E = 4, 2048, 8, 64
NCORES = 8
PT = 128          # partition tile (s-block size)
NTILES = L // PT  # 16
BAND = 256        # covers all non-constant bias diagonals (saturation at 113)
QSPAN = 1024      # q-span per PSUM scores tile (2 banks)
NUM_BUCKETS = 32
MAX_DISTANCE = 128

F32 = mybir.dt.float32
F32R = mybir.dt.float32r
AFT = mybir.ActivationFunctionType

_CACHE: dict = {}
LAST_RESULTS = None  # test harness introspection (exec_time_ns etc.)
LABELS: dict = {}    # mybir instruction name -> human label (for audits)
SIM_MAKESPAN_NS = None  # Tile cost-model predicted per-core makespan


def _chunks512(lo: int, hi: int):
    """Split [lo, hi) at absolute multiples of 512 (PSUM bank boundaries)."""
    c = lo
    while c < hi:
        ce = min(hi, (c // 512 + 1) * 512)
        yield c, ce
        c = ce


def _order(after, before):
    """Pin same-engine scheduling order without a semaphore."""
    add_dep_helper(after.ins, before.ins, sync=False, reason="order")


def _lab(bi, label):
    LABELS[bi.ins.name] = label
    return bi


def _build_nc() -> bass.Bass:
    nc = bass.Bass()
    qkT = nc.dram_tensor("qkT", [B, E, 2, L], F32R, kind="ExternalInput")
    # vw[b] = [vhat (NTILES*65 cols) | wband (256 cols, used from b=0 only)]
    VW = NTILES * (E + 1)
    vw = nc.dram_tensor("vw", [B, PT, VW + BAND], F32R, kind="ExternalInput")
    outT = nc.dram_tensor("outT", [B, E + 1, L], F32, kind="ExternalOutput")

    ntiles_b = 24  # scores tiles per batch: 8 j's with 2 spans + 8 with 1

    with tile.TileContext(nc) as tc:
        with (
            tc.tile_pool(name="qk", bufs=2) as qk,
            tc.tile_pool(name="vh", bufs=2) as vh,
            tc.tile_pool(name="pp", bufs=ntiles_b) as pp,
            tc.tile_pool(name="pb", bufs=NTILES) as pb,
            tc.tile_pool(name="dr", bufs=2 * B) as dr,
            tc.tile_pool(name="ps_s", bufs=2, space="PSUM") as ps_s,
            tc.tile_pool(name="ps_o", bufs=1, space="PSUM") as ps_o,
        ):
            o_ps = ps_o.tile([E + 1, L], F32)  # single accumulator, all batches
            wband_sb = None

            # t=0 warmup: preload the exp table set on ACT and ramp the PE
            # p-state with +0 matmuls while the first input DMA is in flight
            warm_sb = vh.tile([E, 512], F32, tag="warm", bufs=1)
            nc.vector.memset(warm_sb[:], 0.0)
            warm_act = vh.tile([1, 1], F32, tag="warm_act", bufs=1)
            nc.scalar.activation(warm_act[:], warm_sb[0:1, 0:1], AFT.Exp)
            for wi in range(5):
                nc.tensor.matmul(
                    o_ps[0:1, 0:256],
                    lhsT=warm_sb[:, 0:1],
                    rhs=warm_sb[:, 0:256],
                    start=False,
                    stop=False,
                    skip_group_check=True,
                )

            for b in range(B):
                qk_sb = qk.tile([E, 2, L], F32R, tag="qk")
                nc.sync.dma_start(out=qk_sb[:, :, 0 : L // 2], in_=qkT[b, :, :, 0 : L // 2])
                nc.sync.dma_start(out=qk_sb[:, :, L // 2 :], in_=qkT[b, :, :, L // 2 :])
                if b == 0:
                    vw_sb = vh.tile([PT, VW + BAND], F32R, tag="vw0", bufs=1)
                    nc.gpsimd.dma_start(out=vw_sb[:], in_=vw[0])
                    wband_sb = vw_sb[:, VW : VW + BAND]
                else:
                    vw_sb = vh.tile([PT, VW], F32R, tag="vw")
                    nc.gpsimd.dma_start(out=vw_sb[:], in_=vw[b, :, :VW])
                vhat_sb = vw_sb[:, :VW].rearrange("p (t c) -> p t c", c=E + 1)
                qT_sb = qk_sb[:, 0, :]
                kT_sb = qk_sb[:, 1, :]

                bank_start = {}  # bank -> its clearing (start=True) matmul
                for j in range(NTILES):
                    s0 = PT * j
                    spans = [(s0, min(s0 + QSPAN, L))]
                    if s0 + QSPAN < L:
                        spans.append((s0 + QSPAN, L))
                    for si, (qa, qb_) in enumerate(spans):
                        w = qb_ - qa
                        s_ps = ps_s.tile([PT, QSPAN], F32, tag="s")
                        for c0 in range(0, w, 512):
                            c1 = min(c0 + 512, w)
                            _lab(
                                nc.tensor.matmul(
                                    s_ps[:, c0:c1],
                                    lhsT=kT_sb[:, s0 : s0 + PT],
                                    rhs=qT_sb[:, qa + c0 : qa + c1],
                                    start=True,
                                    stop=True,
                                ),
                                f"mm1_{b}_{j}_{si}_{c0}",
                            )
                        p_sb = pp.tile([PT, QSPAN], F32R, tag="p")
                        _lab(
                            nc.scalar.activation(
                                p_sb[:, :w],
                                s_ps[:, :w],
                                AFT.Exp,
                                scale=1.0 / math.sqrt(E),
                            ),
                            f"exp_{b}_{j}_{si}",
                        )
                        if qa == s0:
                            bw = min(BAND, w)
                            pband = pb.tile([PT, BAND], F32R, tag="pband")
                            _lab(
                                nc.vector.tensor_mul(
                                    pband[:, :bw], p_sb[:, :bw], wband_sb[:, :bw]
                                ),
                                f"band_{b}_{j}",
                            )
                            rhs_parts = [
                                (c, ce, pband, qa)
                                for c, ce in _chunks512(qa, qa + bw)
                            ] + [
                                (c, ce, p_sb, qa)
                                for c, ce in _chunks512(qa + bw, qb_)
                            ]
                        else:
                            rhs_parts = [
                                (c, ce, p_sb, qa) for c, ce in _chunks512(qa, qb_)
                            ]
                        for c, ce, rhs_tile, base in rhs_parts:
                            bank = c // 512
                            # exactly one start=True per bank per batch: the
                            # bank-clear wipes has_written for the whole bank,
                            # so later same-bank chunks must accumulate AND be
                            # ordered after the clearing chunk
                            is_start = bank not in bank_start
                            mm2 = _lab(
                                nc.tensor.matmul(
                                    o_ps[:, c:ce],
                                    lhsT=vhat_sb[:, j, :],
                                    rhs=rhs_tile[:, c - base : ce - base],
                                    start=is_start,
                                    stop=(j == min(NTILES - 1, 4 * bank + 3)),
                                    skip_group_check=True,
                                ),
                                f"mm2_{b}_{j}_{si}_{c}",
                            )
                            if is_start:
                                bank_start[bank] = mm2
                            else:
                                _order(mm2, bank_start[bank])
                    # drain accumulator halves (banks 0-1 at j=7, 2-3 at j=15)
                    for h in range(2):
                        if j == 8 * h + 7:
                            o_sb = dr.tile([E + 1, 1024], F32, tag="drain")
                            _lab(
                                nc.vector.tensor_copy(
                                    o_sb[:], o_ps[:, 1024 * h : 1024 * (h + 1)]
                                ),
                                f"drain_{b}_{h}",
                            )
                            nc.gpsimd.dma_start(
                                out=outT[b, :, 1024 * h : 1024 * (h + 1)],
                                in_=o_sb[:],
                            )

    _split_multiwaits(nc)
    return nc


def _split_multiwaits(nc) -> None:
    """walrus accepts at most one sync wait per instruction; hoist extra
    waits onto preceding sequencer nops (identical blocking semantics: the
    sequencer processes the nops' waits in order before the instruction)."""
    for bb in nc.main_func.blocks:
        i = 0
        while i < len(bb.instructions):
            ins = bb.instructions[i]
            si = ins.sync_info
            if si is not None and si.on_wait and len(si.on_wait) > 1:
                waits = list(si.on_wait)
                for w in waits[:-1]:
                    nop = mybir.InstNoOp(
                        name=nc.get_next_instruction_name(), ins=[], outs=[]
                    )
                    nop.engine = ins.engine
                    nop.sync_info = mybir.SyncInfo(on_wait=[w], on_update=[])
                    bb.instructions.insert(i, nop)
                    i += 1
                ins.sync_info = mybir.SyncInfo(
                    on_wait=[waits[-1]], on_update=list(si.on_update or [])
                )
            i += 1


def audit(nc) -> dict:
    """Count semaphore waits per scheduled instruction, keyed by type."""
    from collections import defaultdict

    worst = defaultdict(int)
    multi = []
    for bb in nc.main_func.blocks:
        for ins in bb.instructions:
            si = ins.sync_info
            n = len(si.on_wait) if si and si.on_wait else 0
            t = type(ins).__name__
            worst[t] = max(worst[t], n)
            if n > 1:
                multi.append(
                    (
                        ins.name,
                        LABELS.get(ins.name, "?"),
                        t,
                        [(w.ant_name, w.wait_value) for w in si.on_wait],
                    )
                )
    return {"worst": dict(worst), "multi": multi}


def _get_nc() -> bass.Bass:
    if "nc" not in _CACHE:
        global SIM_MAKESPAN_NS
        import concourse.bass_interp as _bi

        sims = []
        _orig = _bi.CoreSim.simulate

        def _patched(self, *a, **kw):
            r = _orig(self, *a, **kw)
            sims.append(self)
            return r

        _bi.CoreSim.simulate = _patched
        try:
            _CACHE["nc"] = _build_nc()
        finally:
            _bi.CoreSim.simulate = _orig
        if sims:
            SIM_MAKESPAN_NS = int(sims[-1]._sim_state.time)
    return _CACHE["nc"]


def _rel_bucket_np(d: np.ndarray) -> np.ndarray:
    """T5 bucket for non-negative distances d = l - s; mirrors the reference."""
    max_exact = NUM_BUCKETS // 2
    is_small = d < max_exact
    dc = np.maximum(d, 1).astype(np.float32)
    # NOTE: the f32->i32 cast in the jax reference rounds-to-nearest on the
    # TRN backend (not truncation like numpy/CPU) — np.rint mirrors that.
    large = max_exact + np.rint(
        np.log(dc / np.float32(max_exact))
        / math.log(MAX_DISTANCE / max_exact)
        * (NUM_BUCKETS - max_exact)
    ).astype(np.int32)
    large = np.minimum(large, NUM_BUCKETS - 1)
    return np.where(is_small, d, large).astype(np.int32)


def kernel(queries, keys, values, bias_table) -> np.ndarray:
    queries = np.ascontiguousarray(np.asarray(queries), dtype=np.float32)
    keys = np.ascontiguousarray(np.asarray(keys), dtype=np.float32)
    values = np.ascontiguousarray(np.asarray(values), dtype=np.float32)
    bias_table = np.ascontiguousarray(np.asarray(bias_table), dtype=np.float32)

    # [B, L, H, E] -> [B, H, E, L], then interleave q/k: qkT[b, e, 0/1, l]
    qT = queries.transpose(0, 2, 3, 1)
    kT = keys.transpose(0, 2, 3, 1)
    qkT = np.stack([qT, kT], axis=3)  # [B, H, E, 2, L]
    # [B, L, H, E] -> per-head [B, PT, NTILES, E] (+ ones column)
    v5 = values.reshape(B, NTILES, PT, H, E).transpose(3, 0, 2, 1, 4)  # [H,B,PT,NT,E]

    # band of exp(bias - bias_31): distance d = t - p on [128, 256]; 0 for d < 0
    p_idx = np.arange(PT)[:, None]
    t_idx = np.arange(BAND)[None, :]
    d = t_idx - p_idx
    buckets = _rel_bucket_np(np.maximum(d, 0))  # [PT, BAND]
    # beyond the band every distance must sit in the saturated bucket
    assert int(_rel_bucket_np(np.array([BAND - PT])).min()) == NUM_BUCKETS - 1

    in_maps = []
    for h in range(H):
        bias_h = bias_table[:, h]  # [32]
        wb = np.exp(bias_h[buckets] - bias_h[NUM_BUCKETS - 1]).astype(np.float32)
        wb[d < 0] = 0.0  # causal mask
        vw_h = np.empty((B, PT, NTILES * (E + 1) + BAND), np.float32)
        vhat_h = vw_h[:, :, : NTILES * (E + 1)].reshape(B, PT, NTILES, E + 1)
        vhat_h[..., :E] = v5[h]
        vhat_h[..., E] = 1.0
        vw_h[:, :, NTILES * (E + 1) :] = wb[None]
        in_maps.append(
            {
                "qkT": np.ascontiguousarray(qkT[:, h]),
                "vw": np.ascontiguousarray(vw_h),
            }
        )

    global LAST_RESULTS
    res = run_bass_kernel_spmd(_get_nc(), in_maps, list(range(NCORES)))
    LAST_RESULTS = res

    out = np.empty((B, L, H, E), np.float32)
    for h in range(NCORES):
        oT = np.asarray(res.results[h]["outT"])  # [B, E+1, L]
        out[:, :, h, :] = (oT[:, :E, :] / oT[:, E : E + 1, :]).transpose(0, 2, 1)
    return out


# revision 32
# speedup vs baseline: 1.0064x; 1.0064x over previous
"""Trainium2 Bass kernel for nn_DoubleAttention (causal attention + T5 relative
position bias), SPMD over 8 NeuronCores (head-parallel: core h owns head h).

Math notes that shape the kernel:
  * scores^T layout [s, q] (s on partitions) lets both attention matmuls run
    without any on-device transpose: QK^T uses host-pretransposed Q^T/K^T
    ([E, L] per head), and P^T is directly the rhs of the AV matmul with V in
    its natural [s, e] layout.
  * The T5 bias is Toeplitz: bias[l, s] = f(l - s), and the bucket function
    saturates at bucket 31 for distance >= 113.  Softmax is invariant to a
    per-row constant shift, so using exp(bias - bias_31) as a multiplicative
    factor makes the factor exactly 1.0 for distance >= 113.  Together with the
    causal mask this collapses to ONE [128, 256] band table applied to the
    first <=256 columns of each P^T tile.
  * Appending a ones-column to V makes the AV matmul also produce the softmax
    denominator (row 64 of the [65, L] accumulator); normalization happens on
    host, so no on-device reduction/divide at all.

Scheduling notes (walrus rejects matmuls carrying >1 semaphore wait):
  * Production AV matmuls are all start=False members of one accumulation
    group per PSUM bank (opened by a tiny start=True dummy per batch), so they
    carry no write-after-write completion self-waits.
  * Tiny "absorber" matmuls that genuinely read/write the tiles in question
    take each cross-engine wait first (absorbed waits update Tile's observed
    vector clock; nop waits do not):
      - dummy_q / dummy_v join the open bank-3 group and absorb the input-DMA
        waits (they accumulate lhsT*0 = +0, numerically harmless);
      - dummy_sp (per scores tile) takes the PSUM-slot WAR wait vs the exp
        that read the slot two tiles earlier;
      - dummy_tail (per batch) takes the completion tick of the batch's last
        AV matmul, covering the next batch's cross-generation writes;
      - dummy_war (per batch) takes the accumulator-WAR-vs-drain tick;
      - one DVE copy (wband) and one ACT exp (dummy_act) absorb ticks their
        engines would otherwise first observe mid-pipeline.
  * The banded product goes to its own tile (pband) so every AV matmul chunk
    reads a tile with exactly one producer (ACT or DVE, never both).
"""

import math

import numpy as np

import concourse.bass as bass
import concourse.mybir as mybir
import concourse.tile as tile
from concourse.bass_utils import run_bass_kernel_spmd
from concourse.tile_rust import add_dep_helper

B, L, H, E = 4, 2048, 8, 64
NCORES = 8
PT = 128          # partition tile (s-block size)
NTILES = L // PT  # 16
BAND = 256        # covers all non-constant bias diagonals (saturation at 113)
QSPAN = 1024      # q-span per PSUM scores tile (2 banks)
NUM_BUCKETS = 32
MAX_DISTANCE = 128

F32 = mybir.dt.float32
F32R = mybir.dt.float32r
AFT = mybir.ActivationFunctionType

_CACHE: dict = {}
LAST_RESULTS = None  # test harness introspection (exec_time_ns etc.)
LABELS: dict = {}    # mybir instruction name -> human label (for audits)
SIM_MAKESPAN_NS = None  # Tile cost-model predicted per-core makespan


def _chunks512(lo: int, hi: int):
    """Split [lo, hi) at absolute multiples of 512 (PSUM bank boundaries)."""
    c = lo
    while c < hi:
        ce = min(hi, (c // 512 + 1) * 512)
        yield c, ce
        c = ce


def _order(after, before):
    """Pin same-engine scheduling order without a semaphore."""
    add_dep_helper(after.ins, before.ins, sync=False, reason="order")


def _lab(bi, label):
    LABELS[bi.ins.name] = label
    return bi


def _build_nc() -> bass.Bass:
    nc = bass.Bass()
    qkT = nc.dram_tensor("qkT", [B, E, 2, L], F32R, kind="ExternalInput")
    # vw[b] = [vhat (NTILES*65 cols) | wband (256 cols, used from b=0 only)]
    VW = NTILES * (E + 1)
    vw = nc.dram_tensor("vw", [B, PT, VW + BAND], F32R, kind="ExternalInput")
    outT = nc.dram_tensor("outT", [B, E + 1, L], F32, kind="ExternalOutput")

    ntiles_b = 24  # scores tiles per batch: 8 j's with 2 spans + 8 with 1

    with tile.TileContext(nc) as tc:
        with (
            tc.tile_pool(name="qk", bufs=2) as qk,
            tc.tile_pool(name="vh", bufs=2) as vh,
            tc.tile_pool(name="pp", bufs=ntiles_b) as pp,
            tc.tile_pool(name="pb", bufs=NTILES) as pb,
            tc.tile_pool(name="dr", bufs=4 * B) as dr,
            tc.tile_pool(name="ps_s", bufs=2, space="PSUM") as ps_s,
            tc.tile_pool(name="ps_o", bufs=1, space="PSUM") as ps_o,
        ):
            o_ps = ps_o.tile([E + 1, L], F32)  # single accumulator, all batches
            wband_sb = None

            # t=0 warmup: preload the exp table set on ACT and ramp the PE
            # p-state with +0 matmuls while the first input DMA is in flight
            warm_sb = vh.tile([E, 512], F32, tag="warm", bufs=1)
            nc.vector.memset(warm_sb[:], 0.0)
            warm_act = vh.tile([1, 1], F32, tag="warm_act", bufs=1)
            nc.scalar.activation(warm_act[:], warm_sb[0:1, 0:1], AFT.Exp)
            for wi in range(5):
                nc.tensor.matmul(
                    o_ps[0:1, 0:256],
                    lhsT=warm_sb[:, 0:1],
                    rhs=warm_sb[:, 0:256],
                    start=False,
                    stop=False,
                    skip_group_check=True,
                )

            for b in range(B):
                qk_sb = qk.tile([E, 2, L], F32R, tag="qk")
                nc.sync.dma_start(out=qk_sb[:, :, 0 : L // 2], in_=qkT[b, :, :, 0 : L // 2])
                nc.sync.dma_start(out=qk_sb[:, :, L // 2 :], in_=qkT[b, :, :, L // 2 :])
                if b == 0:
                    vw_sb = vh.tile([PT, VW + BAND], F32R, tag="vw0", bufs=1)
                    nc.gpsimd.dma_start(out=vw_sb[:], in_=vw[0])
                    wband_sb = vw_sb[:, VW : VW + BAND]
                else:
                    vw_sb = vh.tile([PT, VW], F32R, tag="vw")
                    nc.gpsimd.dma_start(out=vw_sb[:], in_=vw[b, :, :VW])
                vhat_sb = vw_sb[:, :VW].rearrange("p (t c) -> p t c", c=E + 1)
                qT_sb = qk_sb[:, 0, :]
                kT_sb = qk_sb[:, 1, :]

                bank_start = {}  # bank -> its clearing (start=True) matmul
                for j in range(NTILES):
                    s0 = PT * j
                    spans = [(s0, min(s0 + QSPAN, L))]
                    if s0 + QSPAN < L:
                        spans.append((s0 + QSPAN, L))
                    for si, (qa, qb_) in enumerate(spans):
                        w = qb_ - qa
                        s_ps = ps_s.tile([PT, QSPAN], F32, tag="s")
                        for c0 in range(0, w, 512):
                            c1 = min(c0 + 512, w)
                            _lab(
                                nc.tensor.matmul(
                                    s_ps[:, c0:c1],
                                    lhsT=kT_sb[:, s0 : s0 + PT],
                                    rhs=qT_sb[:, qa + c0 : qa + c1],
                                    start=True,
                                    stop=True,
                                ),
                                f"mm1_{b}_{j}_{si}_{c0}",
                            )
                        p_sb = pp.tile([PT, QSPAN], F32R, tag="p")
                        _lab(
                            nc.scalar.activation(
                                p_sb[:, :w],
                                s_ps[:, :w],
                                AFT.Exp,
                                scale=1.0 / math.sqrt(E),
                            ),
                            f"exp_{b}_{j}_{si}",
                        )
                        if qa == s0:
                            bw = min(BAND, w)
                            pband = pb.tile([PT, BAND], F32R, tag="pband")
                            _lab(
                                nc.vector.tensor_mul(
                                    pband[:, :bw], p_sb[:, :bw], wband_sb[:, :bw]
                                ),
                                f"band_{b}_{j}",
                            )
                            rhs_parts = [
                                (c, ce, pband, qa)
                                for c, ce in _chunks512(qa, qa + bw)
                            ] + [
                                (c, ce, p_sb, qa)
                                for c, ce in _chunks512(qa + bw, qb_)
                            ]
                        else:
                            rhs_parts = [
                                (c, ce, p_sb, qa) for c, ce in _chunks512(qa, qb_)
                            ]
                        for c, ce, rhs_tile, base in rhs_parts:
                            bank = c // 512
                            # exactly one start=True per bank per batch: the
                            # bank-clear wipes has_written for the whole bank,
                            # so later same-bank chunks must accumulate AND be
                            # ordered after the clearing chunk
                            is_start = bank not in bank_start
                            mm2 = _lab(
                                nc.tensor.matmul(
                                    o_ps[:, c:ce],
                                    lhsT=vhat_sb[:, j, :],
                                    rhs=rhs_tile[:, c - base : ce - base],
                                    start=is_start,
                                    stop=(j == min(NTILES - 1, 4 * bank + 3)),
                                    skip_group_check=True,
                                ),
                                f"mm2_{b}_{j}_{si}_{c}",
                            )
                            if is_start:
                                bank_start[bank] = mm2
                            else:
                                _order(mm2, bank_start[bank])
                    # drain each accumulator bank right after its last writer
                    for p in range(4):
                        if j == min(NTILES - 1, 4 * p + 3):
                            o_sb = dr.tile([E + 1, 512], F32, tag="drain")
                            _lab(
                                nc.vector.tensor_copy(
                                    o_sb[:], o_ps[:, 512 * p : 512 * (p + 1)]
                                ),
                                f"drain_{b}_{p}",
                            )
                            nc.gpsimd.dma_start(
                                out=outT[b, :, 512 * p : 512 * (p + 1)],
                                in_=o_sb[:],
                            )

    _split_multiwaits(nc)
    return nc


def _split_multiwaits(nc) -> None:
    """walrus accepts at most one sync wait per instruction; hoist extra
    waits onto preceding sequencer nops (identical blocking semantics: the
    sequencer processes the nops' waits in order before the instruction)."""
    for bb in nc.main_func.blocks:
        i = 0
        while i < len(bb.instructions):
            ins = bb.instructions[i]
            si = ins.sync_info
            if si is not None and si.on_wait and len(si.on_wait) > 1:
                waits = list(si.on_wait)
                for w in waits[:-1]:
                    nop = mybir.InstNoOp(
                        name=nc.get_next_instruction_name(), ins=[], outs=[]
                    )
                    nop.engine = ins.engine
                    nop.sync_info = mybir.SyncInfo(on_wait=[w], on_update=[])
                    bb.instructions.insert(i, nop)
                    i += 1
                ins.sync_info = mybir.SyncInfo(
                    on_wait=[waits[-1]], on_update=list(si.on_update or [])
                )
            i += 1


def audit(nc) -> dict:
    """Count semaphore waits per scheduled instruction, keyed by type."""
    from collections import defaultdict

    worst = defaultdict(int)
    multi = []
    for bb in nc.main_func.blocks:
        for ins in bb.instructions:
            si = ins.sync_info
            n = len(si.on_wait) if si and si.on_wait else 0
            t = type(ins).__name__
            worst[t] = max(worst[t], n)
            if n > 1:
                multi.append(
                    (
                        ins.name,
                        LABELS.get(ins.name, "?"),
                        t,
                        [(w.ant_name, w.wait_value) for w in si.on_wait],
                    )
                )
    return {"worst": dict(worst), "multi": multi}


def _get_nc() -> bass.Bass:
    if "nc" not in _CACHE:
        global SIM_MAKESPAN_NS
        import concourse.bass_interp as _bi

        sims = []
        _orig = _bi.CoreSim.simulate

        def _patched(self, *a, **kw):
            r = _orig(self, *a, **kw)
            sims.append(self)
            return r

        _bi.CoreSim.simulate = _patched
        try:
            _CACHE["nc"] = _build_nc()
        finally:
            _bi.CoreSim.simulate = _orig
        if sims:
            SIM_MAKESPAN_NS = int(sims[-1]._sim_state.time)
    return _CACHE["nc"]


def _rel_bucket_np(d: np.ndarray) -> np.ndarray:
    """T5 bucket for non-negative distances d = l - s; mirrors the reference."""
    max_exact = NUM_BUCKETS // 2
    is_small = d < max_exact
    dc = np.maximum(d, 1).astype(np.float32)
    # NOTE: the f32->i32 cast in the jax reference rounds-to-nearest on the
    # TRN backend (not truncation like numpy/CPU) — np.rint mirrors that.
    large = max_exact + np.rint(
        np.log(dc / np.float32(max_exact))
        / math.log(MAX_DISTANCE / max_exact)
        * (NUM_BUCKETS - max_exact)
    ).astype(np.int32)
    large = np.minimum(large, NUM_BUCKETS - 1)
    return np.where(is_small, d, large).astype(np.int32)


def kernel(queries, keys, values, bias_table) -> np.ndarray:
    queries = np.ascontiguousarray(np.asarray(queries), dtype=np.float32)
    keys = np.ascontiguousarray(np.asarray(keys), dtype=np.float32)
    values = np.ascontiguousarray(np.asarray(values), dtype=np.float32)
    bias_table = np.ascontiguousarray(np.asarray(bias_table), dtype=np.float32)

    # [B, L, H, E] -> [B, H, E, L], then interleave q/k: qkT[b, e, 0/1, l]
    qT = queries.transpose(0, 2, 3, 1)
    kT = keys.transpose(0, 2, 3, 1)
    qkT = np.stack([qT, kT], axis=3)  # [B, H, E, 2, L]
    # [B, L, H, E] -> per-head [B, PT, NTILES, E] (+ ones column)
    v5 = values.reshape(B, NTILES, PT, H, E).transpose(3, 0, 2, 1, 4)  # [H,B,PT,NT,E]

    # band of exp(bias - bias_31): distance d = t - p on [128, 256]; 0 for d < 0
    p_idx = np.arange(PT)[:, None]
    t_idx = np.arange(BAND)[None, :]
    d = t_idx - p_idx
    buckets = _rel_bucket_np(np.maximum(d, 0))  # [PT, BAND]
    # beyond the band every distance must sit in the saturated bucket
    assert int(_rel_bucket_np(np.array([BAND - PT])).min()) == NUM_BUCKETS - 1

    in_maps = []
    for h in range(H):
        bias_h = bias_table[:, h]  # [32]
        wb = np.exp(bias_h[buckets] - bias_h[NUM_BUCKETS - 1]).astype(np.float32)
        wb[d < 0] = 0.0  # causal mask
        vw_h = np.empty((B, PT, NTILES * (E + 1) + BAND), np.float32)
        vhat_h = vw_h[:, :, : NTILES * (E + 1)].reshape(B, PT, NTILES, E + 1)
        vhat_h[..., :E] = v5[h]
        vhat_h[..., E] = 1.0
        vw_h[:, :, NTILES * (E + 1) :] = wb[None]
        in_maps.append(
            {
                "qkT": np.ascontiguousarray(qkT[:, h]),
                "vw": np.ascontiguousarray(vw_h),
            }
        )

    global LAST_RESULTS
    res = run_bass_kernel_spmd(_get_nc(), in_maps, list(range(NCORES)))
    LAST_RESULTS = res

    out = np.empty((B, L, H, E), np.float32)
    for h in range(NCORES):
        oT = np.asarray(res.results[h]["outT"])  # [B, E+1, L]
        out[:, :, h, :] = (oT[:, :E, :] / oT[:, E : E + 1, :]).transpose(0, 2, 1)
    return out
